# revision 19
# baseline (speedup 1.0000x reference)
"""Transformer encoder layer (B=4, S=2048, D=512, F=2048) on 8 trn2 NeuronCores.

Sharding: data-parallel over batch (4 batches) x 2-way split over query
positions -> 8 cores, no collectives. Each core computes full K/V for its
batch and 1024 queries end-to-end.

Per-core strategy (fp8 DoubleRow attention + bf16 FFN):
  - q/k/v inputs, Wq/Wk/Wv (pre-scaled x32 on host; 1/32 folded into the
    psum evacuation) are fp8e4m3. All projection/attention matmuls use
    MatmulPerfMode.DoubleRow: lhsT/rhs carry [128, 2, N] APs contracting
    256-deep per instruction at 2x bf16 rate.
  - Layouts keep contraction on partitions with the 2-tile pair on a middle
    free dim: qT8/kT8/vT8 [128, dt, tok], W8 [128, dt, 512], QT8 [128, et, q],
    KT8 [128, et, k], V8 [128, kt, 512], PT8 [128, kt, q].
  - Softmax skips max-subtraction; exp carries bias -ln(4) so fp8 PT values
    stay below e4m3 max even for ~6-sigma scores (the 1/4 cancels between
    numerator and the matmul-ones denominator, which uses the same fp8 PT,
    so normalization is self-consistent).
  - bv is folded into the residual x on the host (softmax rows sum to 1).
  - FFN runs in bf16 (fp8 would put ~3% RMS on the final output): h is
    transposed via PE bf16 transposes, W1/W2 bf16.
  - Engine split (GPSIMD cannot touch PSUM): ACT = exp/relu/V-evac/hT-evac/
    LN2-apply, DVE = QK-evac/LN-stats/LN1-apply/softmax-normalize/residual
    adds, Pool = bf16 h copies + small DMAs.
  - Chunk order software-pipelines LN1+transposes under attention and FFN1
    so the PE never waits on LayerNorm.
  - All DMAs are split into ~64-128KB pieces: a single DMA queue sustains
    only ~22GB/s, so wide spreading sets the head/tail latency.
"""

import sys

for _p in ("/opt/trn_rl_repo",):
    if _p not in sys.path:
        sys.path.append(_p)

import numpy as np
from contextlib import ExitStack

import concourse.bacc as bacc
import concourse.tile as tile
from concourse import mybir
from concourse.bass_utils import run_bass_kernel_spmd

P = 128
B, S, D, F = 4, 2048, 512, 2048
SQ = S // 2          # queries per core
NCORES = 8
EPS = 1e-5
F32 = mybir.dt.float32
BF16 = mybir.dt.bfloat16
FP8 = mybir.dt.float8e4
AF = mybir.ActivationFunctionType
ALU = mybir.AluOpType
DR = mybir.MatmulPerfMode.DoubleRow

DT = D // P          # 4  d tiles
ET = D // P          # 4  e tiles
NKT = S // P         # 16 key-token tiles
KC = S // 512        # 4  key chunks of 512
QC = SQ // 512       # 2  query chunks of 512
QS = SQ // P         # 8  query subtiles of 128
FT = F // P          # 16 f tiles

INV_SQRT_D = 1.0 / float(np.sqrt(D))
WSCALE = 32.0        # host pre-scales fp8 weights (avoids e4m3 subnormals)
EXP_BIAS = -float(np.log(4.0))   # keeps exp() under e4m3 max

_PROGRAM_CACHE = {}
DEBUG_TAPS = False


def _build(need_bqk: bool, need_gb1: bool, need_b2: bool, need_gb2: bool):
    nc = bacc.Bacc()

    qT_d = nc.declare_dram_parameter("qT8", [P, DT * SQ], FP8, isOutput=False)
    kT_d = nc.declare_dram_parameter("kT8", [P, DT * S], FP8, isOutput=False)
    vT_d = nc.declare_dram_parameter("vT8", [P, DT * S], FP8, isOutput=False)
    W8T = 12 if need_bqk else 8          # [wq|M, (wk), wv] tile count
    w8_d = nc.declare_dram_parameter("w8", [P, W8T * 512], FP8, isOutput=False)
    x_d = nc.declare_dram_parameter("x", [P, QS * D], BF16, isOutput=False)
    w1_d = nc.declare_dram_parameter("w1", [P, DT * F], BF16, isOutput=False)
    w2_d = nc.declare_dram_parameter("w2", [P, FT * D], BF16, isOutput=False)
    if need_bqk:
        bqk_d = nc.declare_dram_parameter("bqk", [P, 2 * ET], F32, isOutput=False)
    b1_d = nc.declare_dram_parameter("b1", [P, FT], F32, isOutput=False)
    ident_d = nc.declare_dram_parameter("ident", [P, P], BF16, isOutput=False)
    out_d = nc.declare_dram_parameter("out", [SQ, D], F32, isOutput=True)
    if need_gb1:
        g1_d = nc.declare_dram_parameter("g1", [D], F32, isOutput=False)
        be1_d = nc.declare_dram_parameter("be1", [D], F32, isOutput=False)
    if need_b2:
        b2_d = nc.declare_dram_parameter("b2", [D], F32, isOutput=False)
    if need_gb2:
        g2_d = nc.declare_dram_parameter("g2", [D], F32, isOutput=False)
        be2_d = nc.declare_dram_parameter("be2", [D], F32, isOutput=False)

    with tile.TileContext(nc) as tc, ExitStack() as ctx:
        const = ctx.enter_context(tc.tile_pool(name="const", bufs=1))
        psum = ctx.enter_context(tc.tile_pool(name="psum", bufs=1, space="PSUM"))

        ident_sb = const.tile([P, P], BF16, name="ident_sb")
        ones8 = const.tile([P, 4], FP8, name="ones8")
        nc.vector.memset(ones8, 1.0)
        eps_t = const.tile([P, 1], F32, name="eps_t")
        nc.vector.memset(eps_t, EPS)
        expb_t = const.tile([P, 1], F32, name="expb_t")
        nc.vector.memset(expb_t, EXP_BIAS)
        if need_bqk:
            bqk_sb = const.tile([P, 2 * ET], F32, name="bqk_sb")
        b1_sb = const.tile([P, FT], F32, name="b1_sb")
        warm = const.tile([P, 512], BF16, name="warm")
        nc.vector.memset(warm, 0.0)

        def bcast_row(src_ap, nm):
            row = const.tile([1, D], F32, name=f"{nm}_row")
            nc.gpsimd.dma_start(out=row[:], in_=src_ap[None, :])
            rowb = const.tile([1, D], BF16, name=f"{nm}_rowb")
            nc.scalar.activation(rowb[:], row[:], AF.Copy)
            onesrow = const.tile([1, P], F32, name=f"{nm}_of")
            nc.vector.memset(onesrow, 1.0)
            onesrow_b = const.tile([1, P], BF16, name=f"{nm}_or")
            nc.scalar.activation(onesrow_b[:], onesrow[:], AF.Copy)
            ps_b = psum.tile([P, D], F32, name=f"ps_{nm}", tag="mm", bufs=4)
            nc.tensor.matmul(ps_b[:], onesrow_b[:], rowb[:], start=True, stop=True)
            full = const.tile([P, D], F32, name=f"{nm}_full")
            nc.scalar.activation(full[:], ps_b[:], AF.Copy)
            return full

        # ---- long-lived h tiles ----
        hpool = ctx.enter_context(tc.tile_pool(name="hpool", bufs=1))
        h_res = [hpool.tile([P, D], F32, name=f"h{i}") for i in range(QS)]
        h_bf = [hpool.tile([P, D], BF16, name=f"hb{i}") for i in range(QS)]

        def layer_norm_emit(pool, y, out_tile, g_full, be_full, key, dve_apply):
            """y: [128, D] fp32 tile -> out_tile = LN(y) (*g +be)."""
            stats = pool.tile([P, 6], F32, name=f"st_{key}", tag="st", bufs=4)
            nc.vector.bn_stats(out=stats[:], in_=y[:])
            mv = pool.tile([P, 2], F32, name=f"mv_{key}", tag="mv", bufs=4)
            nc.vector.bn_aggr(out=mv[:], in_=stats[:])
            std = pool.tile([P, 1], F32, name=f"sd_{key}", tag="sd", bufs=4)
            nc.scalar.activation(std[:], mv[:, 1:2], AF.Sqrt, bias=eps_t[:])
            rstd = pool.tile([P, 1], F32, name=f"rs_{key}", tag="rs", bufs=4)
            nc.vector.reciprocal(rstd[:], std[:])
            nmr = pool.tile([P, 1], F32, name=f"nm_{key}", tag="nm", bufs=4)
            nc.vector.tensor_mul(nmr[:], mv[:, 0:1], rstd[:])
            nc.vector.tensor_scalar_mul(nmr[:], nmr[:], -1.0)
            if g_full is None:
                if dve_apply:
                    nc.vector.tensor_scalar(
                        out_tile[:], y[:], rstd[:], nmr[:], ALU.mult, ALU.add
                    )
                else:
                    nc.scalar.activation(
                        out_tile[:], y[:], AF.Identity, bias=nmr[:], scale=rstd[:]
                    )
            else:
                t = pool.tile([P, D], F32, name=f"lt_{key}", tag="lt", bufs=2)
                nc.scalar.activation(t[:], y[:], AF.Identity, bias=nmr[:], scale=rstd[:])
                nc.vector.tensor_mul(t[:], t[:], g_full[:])
                nc.vector.tensor_add(out_tile[:], t[:], be_full[:])

        # ---- FFN weights (persistent; DMAs issued last in queue order) ----
        w12 = ctx.enter_context(tc.tile_pool(name="w12", bufs=1))
        w1_sb = w12.tile([P, DT * F], BF16, name="w1_sb")
        w2_sb = w12.tile([P, FT * D], BF16, name="w2_sb")

        # ---- attention-era working set (lives to the end; LIFO-safe) ----
        era_a = tc.tile_pool(name="era_a", bufs=1)
        ea = era_a.__enter__()
        QT8 = ea.tile([P, ET * SQ], FP8, name="QT8")
        KT8 = ea.tile([P, ET * S], FP8, name="KT8") if need_bqk else None
        V8 = ea.tile([P, NKT * 2 * 257], FP8, name="V8")
        PT8 = [ea.tile([P, NKT * 512], FP8, name=f"PT8_{qc}") for qc in range(QC)]
        x_sb = ea.tile([P, QS * D], BF16, name="x_sb")
        y_tiles = [ea.tile([P, D], F32, name=f"y{qs}") for qs in range(QS)]

        # ---- projection inputs (freed before the FFN era) ----
        era_in = tc.tile_pool(name="era_in", bufs=1)
        ei = era_in.__enter__()
        w8_sb = ei.tile([P, W8T * 512], FP8, name="w8_sb")
        qT_sb = ei.tile([P, DT * SQ], FP8, name="qT_sb")
        kT_sb = ei.tile([P, DT * S], FP8, name="kT_sb")
        vT_sb = ei.tile([P, DT * S], FP8, name="vT_sb")

        # 3-D views: [partition, tile-pair axis, free]
        w8r = w8_sb.rearrange("p (a e) -> p a e", a=W8T)
        WVOF = 8 if need_bqk else 4          # wv tile offset in w8
        qTr = qT_sb.rearrange("p (a q) -> p a q", a=DT)
        kTr = kT_sb.rearrange("p (a k) -> p a k", a=DT)
        vTr = vT_sb.rearrange("p (a k) -> p a k", a=DT)
        QTr = QT8.rearrange("p (a q) -> p a q", a=ET)
        KTr = KT8.rearrange("p (a k) -> p a k", a=ET) if need_bqk else kTr
        V8r = V8.rearrange("p (a h e) -> p a h e", a=NKT, h=2)
        nc.vector.memset(V8r[:, :, :, 0:1], 1.0)
        PTr = [PT8[qc].rearrange("p (a q) -> p a q", a=NKT) for qc in range(QC)]
        onesr = ones8.rearrange("p (a t) -> p a t", a=2)
        xr = x_sb.rearrange("p (a e) -> p a e", a=QS)

        # ---- DMA issue: one start per tensor (issue costs ~0.6us on the
        # sequencer, so fewer+bigger wins); w8 rides the DVE queue so the
        # first two transfers issue in parallel. Small constants on gpsimd.
        # w8 layout: wq tiles 0:4, wk 4:8, wv 8:12.
        nc.scalar.dma_start(out=w8_sb[:], in_=w8_d[:, :])
        HQ = DT * SQ // 2
        nc.sync.dma_start(out=qT_sb[:, :HQ], in_=qT_d[:, :HQ])
        nc.sync.dma_start(out=qT_sb[:, HQ:], in_=qT_d[:, HQ:])
        nc.sync.dma_start(out=kT_sb[:], in_=kT_d[:, :])
        nc.sync.dma_start(out=vT_sb[:], in_=vT_d[:, :])
        nc.gpsimd.dma_start(out=ident_sb[:], in_=ident_d[:, :])
        if need_bqk:
            nc.gpsimd.dma_start(out=bqk_sb[:], in_=bqk_d[:, :])
        nc.gpsimd.dma_start(out=b1_sb[:], in_=b1_d[:, :])

        g1_full = be1_full = b2_full = g2_full = be2_full = None
        if need_gb1:
            g1_full = bcast_row(g1_d, "g1")
            be1_full = bcast_row(be1_d, "be1")
        if need_b2:
            b2_full = bcast_row(b2_d, "b2")
        if need_gb2:
            g2_full = bcast_row(g2_d, "g2")
            be2_full = bcast_row(be2_d, "be2")

        IW = 1.0 / WSCALE

        # ---- PE warmup: data-independent matmuls ramp the clock while the
        # first input DMAs are in flight ----
        ps_w = psum.tile([P, 512], F32, name="ps_w", tag="mm", bufs=4)
        for wi in range(8):
            nc.tensor.matmul(
                ps_w[:], warm[:, 0:P], warm[:], start=(wi == 0), stop=(wi == 7),
            )

        # ---- Q projection: out [128e, 512q] = sum_d Wq'[d,e]^T qT[d,q],
        # where Wq' = Wq@Wk^T in the fast path (K-projection eliminated) ----
        for qc in range(QC):
            for e in range(ET):
                ps = psum.tile([P, 512], F32, name=f"ps_q{e}_{qc}", tag="mm", bufs=4)
                for i in range(DT // 2):
                    nc.tensor.matmul(
                        ps[:],
                        w8r[:, 2 * i : 2 * i + 2, e * P : (e + 1) * P],
                        qTr[:, 2 * i : 2 * i + 2, qc * 512 : (qc + 1) * 512],
                        start=(i == 0),
                        stop=(i == DT // 2 - 1),
                        perf_mode=DR,
                    )
                dst = QTr[:, e, qc * 512 : (qc + 1) * 512]
                if need_bqk:
                    nc.vector.tensor_scalar(
                        dst, ps[:], IW, bqk_sb[:, e : e + 1], ALU.mult, ALU.add
                    )
                else:
                    nc.vector.tensor_scalar_mul(dst, ps[:], IW)
        # ---- K projection (only when biases force the explicit form) ----
        if need_bqk:
            for e in range(ET):
                for kc in range(KC):
                    ps = psum.tile([P, 512], F32, name=f"ps_k{e}_{kc}", tag="mm", bufs=4)
                    for i in range(DT // 2):
                        nc.tensor.matmul(
                            ps[:],
                            w8r[:, 4 + 2 * i : 4 + 2 * i + 2, e * P : (e + 1) * P],
                            kTr[:, 2 * i : 2 * i + 2, kc * 512 : (kc + 1) * 512],
                            start=(i == 0),
                            stop=(i == DT // 2 - 1),
                            perf_mode=DR,
                        )
                    nc.vector.tensor_scalar(
                        KTr[:, e, kc * 512 : (kc + 1) * 512],
                        ps[:], IW, bqk_sb[:, ET + e : ET + e + 1], ALU.mult, ALU.add,
                    )
        # ---- V projection: out [128tok, 512e] = sum_d vT[d,tok]^T Wv[d,e] ----
        for kt in range(NKT):
            ps = psum.tile([P, 512], F32, name=f"ps_v{kt}", tag="mm", bufs=4)
            for i in range(DT // 2):
                nc.tensor.matmul(
                    ps[:],
                    vTr[:, 2 * i : 2 * i + 2, kt * P : (kt + 1) * P],
                    w8r[:, WVOF + 2 * i : WVOF + 2 * i + 2, :],
                    start=(i == 0),
                    stop=(i == DT // 2 - 1),
                    perf_mode=DR,
                )
            nc.scalar.mul(
                V8r[:, kt, :, 1:257],
                ps[:].rearrange("p (h e) -> p h e", h=2),
                IW,
            )

        # x/w1/w2 are needed much later; issuing here keeps their descriptors
        # out of the head-critical DMA rings
        nc.sync.dma_start(out=x_sb[:], in_=x_d[:, :])
        nc.sync.dma_start(out=w1_sb[:], in_=w1_d[:, :])
        nc.sync.dma_start(out=w2_sb[:], in_=w2_d[:, :])

        # ---- scores + attention, per 512-query chunk ----
        def scores_chunk(qc):
            for kt in range(NKT):
                ps = psum.tile([P, 512], F32, name=f"ps_s{kt}_{qc}", tag="mm", bufs=4)
                for i in range(ET // 2):
                    nc.tensor.matmul(
                        ps[:],
                        KTr[:, 2 * i : 2 * i + 2, kt * P : (kt + 1) * P],
                        QTr[:, 2 * i : 2 * i + 2, qc * 512 : (qc + 1) * 512],
                        start=(i == 0),
                        stop=(i == ET // 2 - 1),
                        perf_mode=DR,
                    )
                nc.scalar.activation(
                    PTr[qc][:, kt, :], ps[:], AF.Exp, scale=INV_SQRT_D, bias=expb_t[:]
                )

        def attn_chunk(qc):
            for q4 in range(4):
                qs = qc * 4 + q4
                ps_h = [
                    psum.tile([P, 257], F32, name=f"ps_a{qs}_{h}", tag="mm", bufs=4)
                    for h in range(2)
                ]
                for h in range(2):
                    for i in range(NKT // 2):
                        nc.tensor.matmul(
                            ps_h[h][:],
                            PTr[qc][:, 2 * i : 2 * i + 2, q4 * P : (q4 + 1) * P],
                            V8r[:, 2 * i : 2 * i + 2, h, :],
                            start=(i == 0),
                            stop=(i == NKT // 2 - 1),
                            perf_mode=DR,
                        )
                recip = ea.tile([P, 1], F32, name=f"rc{qs}", tag="rc", bufs=4)
                nc.vector.reciprocal(recip[:], ps_h[0][:, 0:1])
                y = y_tiles[qs]
                nc.vector.tensor_scalar_mul(y[:, 0:256], ps_h[0][:, 1:257], recip[:])
                nc.vector.tensor_scalar_mul(y[:, 256:512], ps_h[1][:, 1:257], recip[:])
                nc.vector.tensor_add(y[:], y[:], xr[:, qs, :])

        # LN1 split: stats emitted early (DVE-only), apply after attention's
        # DVE work so the ACT sqrt never head-of-line-blocks psum evacuation.
        ln_mv = [ea.tile([P, 8], F32, name=f"mv_c{qc}") for qc in range(QC)]
        ln_rs = [ea.tile([P, 8], F32, name=f"rs_c{qc}") for qc in range(QC)]

        def ln_stats_chunk(qc):
            mvb = ln_mv[qc]
            for q4 in range(4):
                qs = qc * 4 + q4
                stats = ea.tile([P, 6], F32, name=f"st_h{qs}", tag="st", bufs=4)
                nc.vector.bn_stats(out=stats[:], in_=y_tiles[qs][:])
                nc.vector.bn_aggr(out=mvb[:, 2 * q4 : 2 * q4 + 2], in_=stats[:])

        def ln_apply_chunk(qc):
            mvb, rsb = ln_mv[qc], ln_rs[qc]
            mvr = mvb.rearrange("p (a t) -> p a t", a=4)
            rsr = rsb.rearrange("p (a t) -> p a t", a=4)
            nc.scalar.activation(rsr[:, :, 1:2], mvr[:, :, 1:2], AF.Sqrt, bias=eps_t[:])
            nc.vector.reciprocal(rsr[:, :, 1:2], rsr[:, :, 1:2])
            nc.vector.tensor_mul(rsr[:, :, 0:1], mvr[:, :, 0:1], rsr[:, :, 1:2])
            nc.vector.tensor_scalar_mul(rsr[:, :, 0:1], rsr[:, :, 0:1], -1.0)
            for q4 in range(4):
                qs = qc * 4 + q4
                nc.vector.tensor_scalar(
                    h_res[qs][:], y_tiles[qs][:],
                    rsb[:, 2 * q4 + 1 : 2 * q4 + 2],
                    rsb[:, 2 * q4 : 2 * q4 + 1],
                    ALU.mult, ALU.add,
                )
                nc.gpsimd.tensor_copy(h_bf[qs][:], h_res[qs][:])

        def ln_chunk(qc):
            if g1_full is None:
                ln_stats_chunk(qc)
                ln_apply_chunk(qc)
                return
            for q4 in range(4):
                qs = qc * 4 + q4
                layer_norm_emit(
                    ea, y_tiles[qs], h_res[qs], g1_full, be1_full, f"h{qs}",
                    dve_apply=True,
                )
                nc.gpsimd.tensor_copy(h_bf[qs][:], h_res[qs][:])

        scores_chunk(0)
        attn_chunk(0)
        scores_chunk(1)
        if g1_full is None:
            ln_stats_chunk(0)
            attn_chunk(1)
            ln_apply_chunk(0)
            ln_stats_chunk(1)
        else:
            ln_chunk(0)
            attn_chunk(1)

        if DEBUG_TAPS:
            dbg = {
                "dbg_QT8": (QT8, FP8, [P, ET * SQ]),
                "dbg_V8": (V8, FP8, [P, NKT * 2 * 257]),
                "dbg_PT0": (PT8[0], FP8, [P, NKT * 512]),
                "dbg_y0": (y_tiles[0], F32, [P, D]),
            }
            for nm, (t, dt_, shp) in dbg.items():
                d = nc.declare_dram_parameter(nm, shp, dt_, isOutput=True)
                nc.sync.dma_start(out=d[:, :], in_=t[:])

        era_in.__exit__(None, None, None)

        # ---- FFN era (pools stacked above era_a, freed in LIFO order) ----
        ffp_cm = tc.tile_pool(name="ffp", bufs=1)
        ffp = ffp_cm.__enter__()
        hT = [ffp.tile([P, DT * 512], BF16, name=f"hT{qc}") for qc in range(QC)]
        hTr = [hT[qc].rearrange("p (a q) -> p a q", a=DT) for qc in range(QC)]
        fT = [ffp.tile([P, FT * 512], BF16, name=f"fT{qc}") for qc in range(QC)]
        fTr = [fT[qc].rearrange("p (a q) -> p a q", a=FT) for qc in range(QC)]
        w1r = w1_sb.rearrange("p (a f) -> p a f", a=DT)
        w2r = w2_sb.rearrange("p (a e) -> p a e", a=FT)
        lnpb_cm = tc.tile_pool(name="lnpb", bufs=1)
        lnpb = lnpb_cm.__enter__()

        def transp_chunk(qc):
            # d-major so hT[d=0] completes first (FFN1 consumes d ascending);
            # 4 transposes land in one psum tile -> single 512-wide evac
            for dp in range(DT // 2):
                ps_t = psum.tile(
                    [P, 8 * P], BF16, name=f"ps_t{qc}_{dp}", tag="tr", bufs=2,
                )
                for j in range(8):
                    d, q4 = 2 * dp + j // 4, j % 4
                    qs = qc * 4 + q4
                    nc.tensor.transpose(
                        ps_t[:, (j // 4) * 512 + q4 * P : (j // 4) * 512 + (q4 + 1) * P],
                        h_bf[qs][:, d * P : (d + 1) * P],
                        ident_sb[:],
                    )
                if dp == 0:
                    nc.scalar.copy(hTr[qc][:, 2 * dp : 2 * dp + 2, :], ps_t[:])
                else:
                    nc.vector.tensor_copy(hTr[qc][:, 2 * dp : 2 * dp + 2, :], ps_t[:])

        def ffn1_chunk(qc):
            for f in range(FT):
                ps = psum.tile([P, 512], F32, name=f"ps_f{f}_{qc}", tag="mm", bufs=4)
                for d in range(DT):
                    nc.tensor.matmul(
                        ps[:],
                        w1r[:, d, f * P : (f + 1) * P],
                        hTr[qc][:, d, :],
                        start=(d == 0),
                        stop=(d == DT - 1),
                    )
                nc.scalar.activation(
                    fTr[qc][:, f, :], ps[:], AF.Relu, bias=b1_sb[:, f : f + 1]
                )

        def ffn2_chunk(qc):
            for q4 in range(4):
                qs = qc * 4 + q4
                ps = psum.tile([P, D], F32, name=f"ps_o{qs}", tag="mm", bufs=4)
                for f in range(FT):
                    nc.tensor.matmul(
                        ps[:],
                        fTr[qc][:, f, q4 * P : (q4 + 1) * P],
                        w2r[:, f, :],
                        start=(f == 0),
                        stop=(f == FT - 1),
                    )
                y2 = lnpb.tile([P, D], F32, name=f"y2_{qs}", tag="y2", bufs=3)
                nc.vector.tensor_add(y2[:], ps[:], h_res[qs][:])
                if b2_full is not None:
                    nc.vector.tensor_add(y2[:], y2[:], b2_full[:])
                out_t = lnpb.tile([P, D], F32, name=f"ot{qs}", tag="ot", bufs=3)
                layer_norm_emit(
                    lnpb, y2, out_t, g2_full, be2_full, f"o{qs}", dve_apply=False
                )
                nc.sync.dma_start(out=out_d[qs * P : (qs + 1) * P, :], in_=out_t[:])

        transp_chunk(0)
        if g1_full is None:
            ln_apply_chunk(1)
        else:
            ln_chunk(1)
        ffn1_chunk(0)
        transp_chunk(1)
        ffn2_chunk(0)
        ffn1_chunk(1)
        ffn2_chunk(1)

        lnpb_cm.__exit__(None, None, None)
        ffp_cm.__exit__(None, None, None)
        era_a.__exit__(None, None, None)

    nc.compile()
    return nc


def _get_program(need_bqk, need_gb1, need_b2, need_gb2):
    key = (need_bqk, need_gb1, need_b2, need_gb2)
    if key not in _PROGRAM_CACHE:
        _PROGRAM_CACHE[key] = _build(*key)
    return _PROGRAM_CACHE[key]


def _to_tiled(a, ntiles):
    """[R, C] with R = ntiles*128 -> [128, ntiles*C] laid out [p, tile, c]."""
    r, c = a.shape
    return np.ascontiguousarray(
        a.reshape(ntiles, P, c).transpose(1, 0, 2).reshape(P, ntiles * c)
    )


def kernel(
    q, k, v, x, Wq, bq, Wk, bk, Wv, bv, g1, be1, W1, b1, W2, b2, g2, be2, _trace=False
):
    q = np.asarray(q, dtype=np.float32)
    k = np.asarray(k, dtype=np.float32)
    v = np.asarray(v, dtype=np.float32)
    x = np.asarray(x, dtype=np.float32)

    need_bqk = bool(np.any(np.asarray(bq) != 0.0) or np.any(np.asarray(bk) != 0.0))
    need_gb1 = bool(np.any(np.asarray(g1) != 1.0) or np.any(np.asarray(be1) != 0.0))
    need_b2 = bool(np.any(np.asarray(b2) != 0.0))
    need_gb2 = bool(np.any(np.asarray(g2) != 1.0) or np.any(np.asarray(be2) != 0.0))

    nc = _get_program(need_bqk, need_gb1, need_b2, need_gb2)

    np_fp8 = mybir.dt.np(FP8)
    np_bf16 = mybir.dt.np(BF16)

    def wprep(W):
        # [D, D] -> [128, dt, 512] fp8, pre-scaled
        return (
            np.asarray(W, np.float32).reshape(DT, P, D).transpose(1, 0, 2)
            * WSCALE
        ).astype(np_fp8).reshape(P, DT * D)

    if need_bqk:
        w8 = np.concatenate([wprep(Wq), wprep(Wk), wprep(Wv)], axis=1)
    else:
        M = np.asarray(Wq, np.float64) @ np.asarray(Wk, np.float64).T
        w8 = np.concatenate([wprep(M.astype(np.float32)), wprep(Wv)], axis=1)
    w1h = (
        np.asarray(W1, np.float32).reshape(DT, P, F).transpose(1, 0, 2)
    ).astype(np_bf16).reshape(P, DT * F)
    w2h = (
        np.asarray(W2, np.float32).reshape(FT, P, D).transpose(1, 0, 2)
    ).astype(np_bf16).reshape(P, FT * D)
    bqk = np.concatenate(
        [
            np.asarray(bq, np.float32).reshape(ET, P).T,
            np.asarray(bk, np.float32).reshape(ET, P).T,
        ],
        axis=1,
    )
    b1h = np.ascontiguousarray(np.asarray(b1, np.float32).reshape(FT, P).T)

    shared = {
        "w8": np.ascontiguousarray(w8),
        "w1": np.ascontiguousarray(w1h),
        "w2": np.ascontiguousarray(w2h),
        "b1": b1h,
        "ident": np.eye(P, dtype=np.float32).astype(np_bf16),
    }
    if need_bqk:
        shared["bqk"] = np.ascontiguousarray(bqk)
    if need_gb1:
        shared["g1"] = np.ascontiguousarray(g1, dtype=np.float32)
        shared["be1"] = np.ascontiguousarray(be1, dtype=np.float32)
    if need_b2:
        shared["b2"] = np.ascontiguousarray(b2, dtype=np.float32)
    if need_gb2:
        shared["g2"] = np.ascontiguousarray(g2, dtype=np.float32)
        shared["be2"] = np.ascontiguousarray(be2, dtype=np.float32)

    bv32 = np.asarray(bv, dtype=np.float32)
    in_maps = []
    for c in range(NCORES):
        b, half = c // 2, c % 2
        sl = slice(half * SQ, (half + 1) * SQ)
        # feature-major [D, tokens] -> [128, dt, tokens] fp8
        qT = _to_tiled(np.ascontiguousarray(q[b, sl].T), DT).astype(np_fp8)
        kT = _to_tiled(np.ascontiguousarray(k[b].T), DT).astype(np_fp8)
        vT = _to_tiled(np.ascontiguousarray(v[b].T), DT).astype(np_fp8)
        xh = _to_tiled(x[b, sl] + bv32[None, :], QS).astype(np_bf16)
        in_maps.append(
            {"qT8": qT, "kT8": kT, "vT8": vT, "x": xh, **shared}
        )

    res = run_bass_kernel_spmd(nc, in_maps, list(range(NCORES)), trace=_trace)

    out = np.empty((B, S, D), dtype=np.float32)
    for c in range(NCORES):
        b, half = c // 2, c % 2
        out[b, half * SQ : (half + 1) * SQ] = res.results[c]["out"]
    if _trace:
        return out, res
    return out


# revision 20
# speedup vs baseline: 1.0340x; 1.0340x over previous
"""Transformer encoder layer (B=4, S=2048, D=512, F=2048) on 8 trn2 NeuronCores.

Sharding: data-parallel over batch (4 batches) x 2-way split over query
positions -> 8 cores, no collectives. Each core computes full K/V for its
batch and 1024 queries end-to-end.

Per-core strategy (fp8 DoubleRow attention + bf16 FFN):
  - q/k/v inputs, Wq/Wk/Wv (pre-scaled x32 on host; 1/32 folded into the
    psum evacuation) are fp8e4m3. All projection/attention matmuls use
    MatmulPerfMode.DoubleRow: lhsT/rhs carry [128, 2, N] APs contracting
    256-deep per instruction at 2x bf16 rate.
  - Layouts keep contraction on partitions with the 2-tile pair on a middle
    free dim: qT8/kT8/vT8 [128, dt, tok], W8 [128, dt, 512], QT8 [128, et, q],
    KT8 [128, et, k], V8 [128, kt, 512], PT8 [128, kt, q].
  - Softmax skips max-subtraction; exp carries bias -ln(4) so fp8 PT values
    stay below e4m3 max even for ~6-sigma scores (the 1/4 cancels between
    numerator and the matmul-ones denominator, which uses the same fp8 PT,
    so normalization is self-consistent).
  - bv is folded into the residual x on the host (softmax rows sum to 1).
  - FFN runs in bf16 (fp8 would put ~3% RMS on the final output): h is
    transposed via PE bf16 transposes, W1/W2 bf16.
  - Engine split (GPSIMD cannot touch PSUM): ACT = exp/relu/V-evac/hT-evac/
    LN2-apply, DVE = QK-evac/LN-stats/LN1-apply/softmax-normalize/residual
    adds, Pool = bf16 h copies + small DMAs.
  - Chunk order software-pipelines LN1+transposes under attention and FFN1
    so the PE never waits on LayerNorm.
  - All DMAs are split into ~64-128KB pieces: a single DMA queue sustains
    only ~22GB/s, so wide spreading sets the head/tail latency.
"""

import sys

for _p in ("/opt/trn_rl_repo",):
    if _p not in sys.path:
        sys.path.append(_p)

import numpy as np
from contextlib import ExitStack

import concourse.bacc as bacc
import concourse.tile as tile
from concourse import mybir
from concourse.bass_utils import run_bass_kernel_spmd

P = 128
B, S, D, F = 4, 2048, 512, 2048
SQ = S // 2          # queries per core
NCORES = 8
EPS = 1e-5
F32 = mybir.dt.float32
BF16 = mybir.dt.bfloat16
FP8 = mybir.dt.float8e4
AF = mybir.ActivationFunctionType
ALU = mybir.AluOpType
DR = mybir.MatmulPerfMode.DoubleRow

DT = D // P          # 4  d tiles
ET = D // P          # 4  e tiles
NKT = S // P         # 16 key-token tiles
KC = S // 512        # 4  key chunks of 512
QC = SQ // 512       # 2  query chunks of 512
QS = SQ // P         # 8  query subtiles of 128
FT = F // P          # 16 f tiles

INV_SQRT_D = 1.0 / float(np.sqrt(D))
WSCALE = 32.0        # host pre-scales fp8 weights (avoids e4m3 subnormals)
EXP_BIAS = -float(np.log(4.0))   # keeps exp() under e4m3 max

_PROGRAM_CACHE = {}
DEBUG_TAPS = False


def _build(need_bqk: bool, need_gb1: bool, need_b2: bool, need_gb2: bool):
    nc = bacc.Bacc()

    qT_d = nc.declare_dram_parameter("qT8", [P, DT * SQ], FP8, isOutput=False)
    kT_d = nc.declare_dram_parameter("kT8", [P, DT * S], FP8, isOutput=False)
    vT_d = nc.declare_dram_parameter("vT8", [P, DT * S], FP8, isOutput=False)
    W8T = 12 if need_bqk else 8          # [wq|M, (wk), wv] tile count
    w8_d = nc.declare_dram_parameter("w8", [P, W8T * 512], FP8, isOutput=False)
    x_d = nc.declare_dram_parameter("x", [P, QS * D], BF16, isOutput=False)
    w1_d = nc.declare_dram_parameter("w1", [P, DT * F], BF16, isOutput=False)
    w2_d = nc.declare_dram_parameter("w2", [P, FT * D], BF16, isOutput=False)
    if need_bqk:
        bqk_d = nc.declare_dram_parameter("bqk", [P, 2 * ET], F32, isOutput=False)
    b1_d = nc.declare_dram_parameter("b1", [P, FT], F32, isOutput=False)
    ident_d = nc.declare_dram_parameter("ident", [P, P], BF16, isOutput=False)
    out_d = nc.declare_dram_parameter("out", [SQ, D], F32, isOutput=True)
    if need_gb1:
        g1_d = nc.declare_dram_parameter("g1", [D], F32, isOutput=False)
        be1_d = nc.declare_dram_parameter("be1", [D], F32, isOutput=False)
    if need_b2:
        b2_d = nc.declare_dram_parameter("b2", [D], F32, isOutput=False)
    if need_gb2:
        g2_d = nc.declare_dram_parameter("g2", [D], F32, isOutput=False)
        be2_d = nc.declare_dram_parameter("be2", [D], F32, isOutput=False)

    with tile.TileContext(nc) as tc, ExitStack() as ctx:
        const = ctx.enter_context(tc.tile_pool(name="const", bufs=1))
        psum = ctx.enter_context(tc.tile_pool(name="psum", bufs=1, space="PSUM"))

        ident_sb = const.tile([P, P], BF16, name="ident_sb")
        ones8 = const.tile([P, 4], FP8, name="ones8")
        nc.vector.memset(ones8, 1.0)
        eps_t = const.tile([P, 1], F32, name="eps_t")
        nc.vector.memset(eps_t, EPS)
        expb_t = const.tile([P, 1], F32, name="expb_t")
        nc.vector.memset(expb_t, EXP_BIAS)
        if need_bqk:
            bqk_sb = const.tile([P, 2 * ET], F32, name="bqk_sb")
        b1_sb = const.tile([P, FT], F32, name="b1_sb")
        warm = const.tile([P, 512], BF16, name="warm")
        nc.vector.memset(warm, 0.0)

        def bcast_row(src_ap, nm):
            row = const.tile([1, D], F32, name=f"{nm}_row")
            nc.gpsimd.dma_start(out=row[:], in_=src_ap[None, :])
            rowb = const.tile([1, D], BF16, name=f"{nm}_rowb")
            nc.scalar.activation(rowb[:], row[:], AF.Copy)
            onesrow = const.tile([1, P], F32, name=f"{nm}_of")
            nc.vector.memset(onesrow, 1.0)
            onesrow_b = const.tile([1, P], BF16, name=f"{nm}_or")
            nc.scalar.activation(onesrow_b[:], onesrow[:], AF.Copy)
            ps_b = psum.tile([P, D], F32, name=f"ps_{nm}", tag="mm", bufs=6)
            nc.tensor.matmul(ps_b[:], onesrow_b[:], rowb[:], start=True, stop=True)
            full = const.tile([P, D], F32, name=f"{nm}_full")
            nc.scalar.activation(full[:], ps_b[:], AF.Copy)
            return full

        # ---- long-lived h tiles ----
        hpool = ctx.enter_context(tc.tile_pool(name="hpool", bufs=1))
        h_res = [hpool.tile([P, D], F32, name=f"h{i}") for i in range(QS)]
        h_bf = [hpool.tile([P, D], BF16, name=f"hb{i}") for i in range(QS)]

        def layer_norm_emit(pool, y, out_tile, g_full, be_full, key, dve_apply):
            """y: [128, D] fp32 tile -> out_tile = LN(y) (*g +be)."""
            stats = pool.tile([P, 6], F32, name=f"st_{key}", tag="st", bufs=4)
            nc.vector.bn_stats(out=stats[:], in_=y[:])
            mv = pool.tile([P, 2], F32, name=f"mv_{key}", tag="mv", bufs=4)
            nc.vector.bn_aggr(out=mv[:], in_=stats[:])
            std = pool.tile([P, 1], F32, name=f"sd_{key}", tag="sd", bufs=4)
            nc.scalar.activation(std[:], mv[:, 1:2], AF.Sqrt, bias=eps_t[:])
            rstd = pool.tile([P, 1], F32, name=f"rs_{key}", tag="rs", bufs=4)
            nc.vector.reciprocal(rstd[:], std[:])
            nmr = pool.tile([P, 1], F32, name=f"nm_{key}", tag="nm", bufs=4)
            nc.vector.tensor_mul(nmr[:], mv[:, 0:1], rstd[:])
            nc.vector.tensor_scalar_mul(nmr[:], nmr[:], -1.0)
            if g_full is None:
                if dve_apply:
                    nc.vector.tensor_scalar(
                        out_tile[:], y[:], rstd[:], nmr[:], ALU.mult, ALU.add
                    )
                else:
                    nc.scalar.activation(
                        out_tile[:], y[:], AF.Identity, bias=nmr[:], scale=rstd[:]
                    )
            else:
                t = pool.tile([P, D], F32, name=f"lt_{key}", tag="lt", bufs=2)
                nc.scalar.activation(t[:], y[:], AF.Identity, bias=nmr[:], scale=rstd[:])
                nc.vector.tensor_mul(t[:], t[:], g_full[:])
                nc.vector.tensor_add(out_tile[:], t[:], be_full[:])

        # ---- FFN weights (persistent; DMAs issued last in queue order) ----
        w12 = ctx.enter_context(tc.tile_pool(name="w12", bufs=1))
        w1_sb = w12.tile([P, DT * F], BF16, name="w1_sb")
        w2_sb = w12.tile([P, FT * D], BF16, name="w2_sb")

        # ---- attention-era working set (lives to the end; LIFO-safe) ----
        era_a = tc.tile_pool(name="era_a", bufs=1)
        ea = era_a.__enter__()
        QT8 = ea.tile([P, ET * SQ], FP8, name="QT8")
        KT8 = ea.tile([P, ET * S], FP8, name="KT8") if need_bqk else None
        V8 = ea.tile([P, NKT * 2 * 257], FP8, name="V8")
        PT8 = [ea.tile([P, NKT * 512], FP8, name=f"PT8_{qc}") for qc in range(QC)]
        x_sb = ea.tile([P, QS * D], BF16, name="x_sb")
        y_tiles = [ea.tile([P, D], F32, name=f"y{qs}") for qs in range(QS)]

        # ---- projection inputs (freed before the FFN era) ----
        era_in = tc.tile_pool(name="era_in", bufs=1)
        ei = era_in.__enter__()
        w8_sb = ei.tile([P, W8T * 512], FP8, name="w8_sb")
        qT_sb = ei.tile([P, DT * SQ], FP8, name="qT_sb")
        kT_sb = ei.tile([P, DT * S], FP8, name="kT_sb")
        vT_sb = ei.tile([P, DT * S], FP8, name="vT_sb")

        # 3-D views: [partition, tile-pair axis, free]
        w8r = w8_sb.rearrange("p (a e) -> p a e", a=W8T)
        WVOF = 8 if need_bqk else 4          # wv tile offset in w8
        qTr = qT_sb.rearrange("p (a q) -> p a q", a=DT)
        kTr = kT_sb.rearrange("p (a k) -> p a k", a=DT)
        vTr = vT_sb.rearrange("p (a k) -> p a k", a=DT)
        QTr = QT8.rearrange("p (a q) -> p a q", a=ET)
        KTr = KT8.rearrange("p (a k) -> p a k", a=ET) if need_bqk else kTr
        V8r = V8.rearrange("p (a h e) -> p a h e", a=NKT, h=2)
        nc.vector.memset(V8r[:, :, :, 0:1], 1.0)
        PTr = [PT8[qc].rearrange("p (a q) -> p a q", a=NKT) for qc in range(QC)]
        onesr = ones8.rearrange("p (a t) -> p a t", a=2)
        xr = x_sb.rearrange("p (a e) -> p a e", a=QS)

        # ---- DMA issue: one start per tensor (issue costs ~0.6us on the
        # sequencer, so fewer+bigger wins); w8 rides the DVE queue so the
        # first two transfers issue in parallel. Small constants on gpsimd.
        # w8 layout: wq tiles 0:4, wk 4:8, wv 8:12.
        nc.scalar.dma_start(out=w8_sb[:], in_=w8_d[:, :])
        HQ = DT * SQ // 2
        nc.sync.dma_start(out=qT_sb[:, :HQ], in_=qT_d[:, :HQ])
        nc.sync.dma_start(out=qT_sb[:, HQ:], in_=qT_d[:, HQ:])
        nc.sync.dma_start(out=kT_sb[:], in_=kT_d[:, :])
        nc.sync.dma_start(out=vT_sb[:], in_=vT_d[:, :])
        nc.gpsimd.dma_start(out=ident_sb[:], in_=ident_d[:, :])
        if need_bqk:
            nc.gpsimd.dma_start(out=bqk_sb[:], in_=bqk_d[:, :])
        nc.gpsimd.dma_start(out=b1_sb[:], in_=b1_d[:, :])

        g1_full = be1_full = b2_full = g2_full = be2_full = None
        if need_gb1:
            g1_full = bcast_row(g1_d, "g1")
            be1_full = bcast_row(be1_d, "be1")
        if need_b2:
            b2_full = bcast_row(b2_d, "b2")
        if need_gb2:
            g2_full = bcast_row(g2_d, "g2")
            be2_full = bcast_row(be2_d, "be2")

        IW = 1.0 / WSCALE

        # ---- PE warmup: data-independent matmuls ramp the clock while the
        # first input DMAs are in flight ----
        ps_w = psum.tile([P, 512], F32, name="ps_w", tag="mm", bufs=6)
        for wi in range(8):
            nc.tensor.matmul(
                ps_w[:], warm[:, 0:P], warm[:], start=(wi == 0), stop=(wi == 7),
            )

        # ---- Q projection: out [128e, 512q] = sum_d Wq'[d,e]^T qT[d,q],
        # where Wq' = Wq@Wk^T in the fast path (K-projection eliminated) ----
        for qc in range(QC):
            for e in range(ET):
                ps = psum.tile([P, 512], F32, name=f"ps_q{e}_{qc}", tag="mm", bufs=6)
                for i in range(DT // 2):
                    nc.tensor.matmul(
                        ps[:],
                        w8r[:, 2 * i : 2 * i + 2, e * P : (e + 1) * P],
                        qTr[:, 2 * i : 2 * i + 2, qc * 512 : (qc + 1) * 512],
                        start=(i == 0),
                        stop=(i == DT // 2 - 1),
                        perf_mode=DR,
                    )
                dst = QTr[:, e, qc * 512 : (qc + 1) * 512]
                if need_bqk:
                    nc.vector.tensor_scalar(
                        dst, ps[:], IW, bqk_sb[:, e : e + 1], ALU.mult, ALU.add
                    )
                else:
                    nc.vector.tensor_scalar_mul(dst, ps[:], IW)
        # ---- K projection (only when biases force the explicit form) ----
        if need_bqk:
            for e in range(ET):
                for kc in range(KC):
                    ps = psum.tile([P, 512], F32, name=f"ps_k{e}_{kc}", tag="mm", bufs=6)
                    for i in range(DT // 2):
                        nc.tensor.matmul(
                            ps[:],
                            w8r[:, 4 + 2 * i : 4 + 2 * i + 2, e * P : (e + 1) * P],
                            kTr[:, 2 * i : 2 * i + 2, kc * 512 : (kc + 1) * 512],
                            start=(i == 0),
                            stop=(i == DT // 2 - 1),
                            perf_mode=DR,
                        )
                    nc.vector.tensor_scalar(
                        KTr[:, e, kc * 512 : (kc + 1) * 512],
                        ps[:], IW, bqk_sb[:, ET + e : ET + e + 1], ALU.mult, ALU.add,
                    )
        # ---- V projection: out [128tok, 512e] = sum_d vT[d,tok]^T Wv[d,e] ----
        for kt in range(NKT):
            ps = psum.tile([P, 512], F32, name=f"ps_v{kt}", tag="mm", bufs=6)
            for i in range(DT // 2):
                nc.tensor.matmul(
                    ps[:],
                    vTr[:, 2 * i : 2 * i + 2, kt * P : (kt + 1) * P],
                    w8r[:, WVOF + 2 * i : WVOF + 2 * i + 2, :],
                    start=(i == 0),
                    stop=(i == DT // 2 - 1),
                    perf_mode=DR,
                )
            nc.scalar.mul(
                V8r[:, kt, :, 1:257],
                ps[:].rearrange("p (h e) -> p h e", h=2),
                IW,
            )

        # x/w1/w2 are needed much later; issuing here keeps their descriptors
        # out of the head-critical DMA rings
        nc.sync.dma_start(out=x_sb[:], in_=x_d[:, :])
        nc.sync.dma_start(out=w1_sb[:], in_=w1_d[:, :])
        nc.sync.dma_start(out=w2_sb[:], in_=w2_d[:, :])

        # ---- scores + attention, per 512-query chunk ----
        def scores_chunk(qc):
            for kt in range(NKT):
                ps = psum.tile([P, 512], F32, name=f"ps_s{kt}_{qc}", tag="mm", bufs=6)
                for i in range(ET // 2):
                    nc.tensor.matmul(
                        ps[:],
                        KTr[:, 2 * i : 2 * i + 2, kt * P : (kt + 1) * P],
                        QTr[:, 2 * i : 2 * i + 2, qc * 512 : (qc + 1) * 512],
                        start=(i == 0),
                        stop=(i == ET // 2 - 1),
                        perf_mode=DR,
                    )
                nc.scalar.activation(
                    PTr[qc][:, kt, :], ps[:], AF.Exp, scale=INV_SQRT_D, bias=expb_t[:]
                )

        def attn_chunk(qc):
            for q4 in range(4):
                qs = qc * 4 + q4
                ps_h = [
                    psum.tile([P, 257], F32, name=f"ps_a{qs}_{h}", tag="mm", bufs=6)
                    for h in range(2)
                ]
                for h in range(2):
                    for i in range(NKT // 2):
                        nc.tensor.matmul(
                            ps_h[h][:],
                            PTr[qc][:, 2 * i : 2 * i + 2, q4 * P : (q4 + 1) * P],
                            V8r[:, 2 * i : 2 * i + 2, h, :],
                            start=(i == 0),
                            stop=(i == NKT // 2 - 1),
                            perf_mode=DR,
                        )
                recip = ea.tile([P, 1], F32, name=f"rc{qs}", tag="rc", bufs=4)
                nc.vector.reciprocal(recip[:], ps_h[0][:, 0:1])
                y = y_tiles[qs]
                nc.vector.tensor_scalar_mul(y[:, 0:256], ps_h[0][:, 1:257], recip[:])
                nc.vector.tensor_scalar_mul(y[:, 256:512], ps_h[1][:, 1:257], recip[:])
                nc.vector.tensor_add(y[:], y[:], xr[:, qs, :])

        def ln_chunk(qc):
            for q4 in range(4):
                qs = qc * 4 + q4
                layer_norm_emit(
                    ea, y_tiles[qs], h_res[qs], g1_full, be1_full, f"h{qs}",
                    dve_apply=True,
                )
                nc.gpsimd.tensor_copy(h_bf[qs][:], h_res[qs][:])

        scores_chunk(0)
        attn_chunk(0)
        scores_chunk(1)
        ln_chunk(0)
        attn_chunk(1)

        if DEBUG_TAPS:
            dbg = {
                "dbg_QT8": (QT8, FP8, [P, ET * SQ]),
                "dbg_V8": (V8, FP8, [P, NKT * 2 * 257]),
                "dbg_PT0": (PT8[0], FP8, [P, NKT * 512]),
                "dbg_y0": (y_tiles[0], F32, [P, D]),
            }
            for nm, (t, dt_, shp) in dbg.items():
                d = nc.declare_dram_parameter(nm, shp, dt_, isOutput=True)
                nc.sync.dma_start(out=d[:, :], in_=t[:])

        era_in.__exit__(None, None, None)

        # ---- FFN era (pools stacked above era_a, freed in LIFO order) ----
        ffp_cm = tc.tile_pool(name="ffp", bufs=1)
        ffp = ffp_cm.__enter__()
        hT = [ffp.tile([P, DT * 512], BF16, name=f"hT{qc}") for qc in range(QC)]
        hTr = [hT[qc].rearrange("p (a q) -> p a q", a=DT) for qc in range(QC)]
        fT = [ffp.tile([P, FT * 512], BF16, name=f"fT{qc}") for qc in range(QC)]
        fTr = [fT[qc].rearrange("p (a q) -> p a q", a=FT) for qc in range(QC)]
        w1r = w1_sb.rearrange("p (a f) -> p a f", a=DT)
        w2r = w2_sb.rearrange("p (a e) -> p a e", a=FT)
        lnpb_cm = tc.tile_pool(name="lnpb", bufs=1)
        lnpb = lnpb_cm.__enter__()

        def transp_chunk(qc):
            # d-major so hT[d=0] completes first (FFN1 consumes d ascending);
            # 4 transposes land in one psum tile -> single 512-wide evac
            for dp in range(DT // 2):
                ps_t = psum.tile(
                    [P, 8 * P], BF16, name=f"ps_t{qc}_{dp}", tag="tr", bufs=2,
                )
                for j in range(8):
                    d, q4 = 2 * dp + j // 4, j % 4
                    qs = qc * 4 + q4
                    nc.tensor.transpose(
                        ps_t[:, (j // 4) * 512 + q4 * P : (j // 4) * 512 + (q4 + 1) * P],
                        h_bf[qs][:, d * P : (d + 1) * P],
                        ident_sb[:],
                    )
                if dp == 0:
                    nc.scalar.copy(hTr[qc][:, 2 * dp : 2 * dp + 2, :], ps_t[:])
                else:
                    nc.vector.tensor_copy(hTr[qc][:, 2 * dp : 2 * dp + 2, :], ps_t[:])

        def ffn1_chunk(qc):
            for f in range(FT):
                ps = psum.tile([P, 512], F32, name=f"ps_f{f}_{qc}", tag="mm", bufs=6)
                for d in range(DT):
                    nc.tensor.matmul(
                        ps[:],
                        w1r[:, d, f * P : (f + 1) * P],
                        hTr[qc][:, d, :],
                        start=(d == 0),
                        stop=(d == DT - 1),
                    )
                nc.scalar.activation(
                    fTr[qc][:, f, :], ps[:], AF.Relu, bias=b1_sb[:, f : f + 1]
                )

        def ffn2_chunk(qc):
            for q4 in range(4):
                qs = qc * 4 + q4
                ps = psum.tile([P, D], F32, name=f"ps_o{qs}", tag="mm", bufs=6)
                for f in range(FT):
                    nc.tensor.matmul(
                        ps[:],
                        fTr[qc][:, f, q4 * P : (q4 + 1) * P],
                        w2r[:, f, :],
                        start=(f == 0),
                        stop=(f == FT - 1),
                    )
                y2 = lnpb.tile([P, D], F32, name=f"y2_{qs}", tag="y2", bufs=3)
                nc.vector.tensor_add(y2[:], ps[:], h_res[qs][:])
                if b2_full is not None:
                    nc.vector.tensor_add(y2[:], y2[:], b2_full[:])
                out_t = lnpb.tile([P, D], F32, name=f"ot{qs}", tag="ot", bufs=3)
                layer_norm_emit(
                    lnpb, y2, out_t, g2_full, be2_full, f"o{qs}", dve_apply=False
                )
                nc.sync.dma_start(out=out_d[qs * P : (qs + 1) * P, :], in_=out_t[:])

        transp_chunk(0)
        ffn1_chunk(0)
        ln_chunk(1)
        transp_chunk(1)
        ffn2_chunk(0)
        ffn1_chunk(1)
        ffn2_chunk(1)

        lnpb_cm.__exit__(None, None, None)
        ffp_cm.__exit__(None, None, None)
        era_a.__exit__(None, None, None)

    nc.compile()
    return nc


def _get_program(need_bqk, need_gb1, need_b2, need_gb2):
    key = (need_bqk, need_gb1, need_b2, need_gb2)
    if key not in _PROGRAM_CACHE:
        _PROGRAM_CACHE[key] = _build(*key)
    return _PROGRAM_CACHE[key]


def _to_tiled(a, ntiles):
    """[R, C] with R = ntiles*128 -> [128, ntiles*C] laid out [p, tile, c]."""
    r, c = a.shape
    return np.ascontiguousarray(
        a.reshape(ntiles, P, c).transpose(1, 0, 2).reshape(P, ntiles * c)
    )


def kernel(
    q, k, v, x, Wq, bq, Wk, bk, Wv, bv, g1, be1, W1, b1, W2, b2, g2, be2, _trace=False
):
    q = np.asarray(q, dtype=np.float32)
    k = np.asarray(k, dtype=np.float32)
    v = np.asarray(v, dtype=np.float32)
    x = np.asarray(x, dtype=np.float32)

    need_bqk = bool(np.any(np.asarray(bq) != 0.0) or np.any(np.asarray(bk) != 0.0))
    need_gb1 = bool(np.any(np.asarray(g1) != 1.0) or np.any(np.asarray(be1) != 0.0))
    need_b2 = bool(np.any(np.asarray(b2) != 0.0))
    need_gb2 = bool(np.any(np.asarray(g2) != 1.0) or np.any(np.asarray(be2) != 0.0))

    nc = _get_program(need_bqk, need_gb1, need_b2, need_gb2)

    np_fp8 = mybir.dt.np(FP8)
    np_bf16 = mybir.dt.np(BF16)

    def wprep(W):
        # [D, D] -> [128, dt, 512] fp8, pre-scaled
        return (
            np.asarray(W, np.float32).reshape(DT, P, D).transpose(1, 0, 2)
            * WSCALE
        ).astype(np_fp8).reshape(P, DT * D)

    if need_bqk:
        w8 = np.concatenate([wprep(Wq), wprep(Wk), wprep(Wv)], axis=1)
    else:
        M = np.asarray(Wq, np.float64) @ np.asarray(Wk, np.float64).T
        w8 = np.concatenate([wprep(M.astype(np.float32)), wprep(Wv)], axis=1)
    w1h = (
        np.asarray(W1, np.float32).reshape(DT, P, F).transpose(1, 0, 2)
    ).astype(np_bf16).reshape(P, DT * F)
    w2h = (
        np.asarray(W2, np.float32).reshape(FT, P, D).transpose(1, 0, 2)
    ).astype(np_bf16).reshape(P, FT * D)
    bqk = np.concatenate(
        [
            np.asarray(bq, np.float32).reshape(ET, P).T,
            np.asarray(bk, np.float32).reshape(ET, P).T,
        ],
        axis=1,
    )
    b1h = np.ascontiguousarray(np.asarray(b1, np.float32).reshape(FT, P).T)

    shared = {
        "w8": np.ascontiguousarray(w8),
        "w1": np.ascontiguousarray(w1h),
        "w2": np.ascontiguousarray(w2h),
        "b1": b1h,
        "ident": np.eye(P, dtype=np.float32).astype(np_bf16),
    }
    if need_bqk:
        shared["bqk"] = np.ascontiguousarray(bqk)
    if need_gb1:
        shared["g1"] = np.ascontiguousarray(g1, dtype=np.float32)
        shared["be1"] = np.ascontiguousarray(be1, dtype=np.float32)
    if need_b2:
        shared["b2"] = np.ascontiguousarray(b2, dtype=np.float32)
    if need_gb2:
        shared["g2"] = np.ascontiguousarray(g2, dtype=np.float32)
        shared["be2"] = np.ascontiguousarray(be2, dtype=np.float32)

    bv32 = np.asarray(bv, dtype=np.float32)
    in_maps = []
    for c in range(NCORES):
        b, half = c // 2, c % 2
        sl = slice(half * SQ, (half + 1) * SQ)
        # feature-major [D, tokens] -> [128, dt, tokens] fp8
        qT = _to_tiled(np.ascontiguousarray(q[b, sl].T), DT).astype(np_fp8)
        kT = _to_tiled(np.ascontiguousarray(k[b].T), DT).astype(np_fp8)
        vT = _to_tiled(np.ascontiguousarray(v[b].T), DT).astype(np_fp8)
        xh = _to_tiled(x[b, sl] + bv32[None, :], QS).astype(np_bf16)
        in_maps.append(
            {"qT8": qT, "kT8": kT, "vT8": vT, "x": xh, **shared}
        )

    res = run_bass_kernel_spmd(nc, in_maps, list(range(NCORES)), trace=_trace)

    out = np.empty((B, S, D), dtype=np.float32)
    for c in range(NCORES):
        b, half = c // 2, c % 2
        out[b, half * SQ : (half + 1) * SQ] = res.results[c]["out"]
    if _trace:
        return out, res
    return out
